# revision 6
# baseline (speedup 1.0000x reference)
"""Embedding lookup (gather) on 8 Trainium2 NeuronCores — bf16 hybrid gather.

Strategy: data-parallel. The [768, 50257] fp32 table is transposed and cast to
bf16 [50257, 768] host-side (max rel err 2^-9 ~ 0.2%, well inside the 2e-2
gate) and replicated to every core's DRAM; the 16384 tokens are sharded 2048
per core and sorted by row index within each core (monotonic HBM addresses,
better DRAM page locality; the host undoes the permutation and casts back to
fp32). Each core gathers its 2048 rows into SBUF and stores them bf16 to its
output shard. No collectives.

The gather bottleneck is SWDGE descriptor generation, and TRN2 offers two
mechanisms with opposite costs:
  - INDIRECT1D (indirect_dma_start): resident in the Pool NX-sequencer
    firmware (no Q7 library), int32 indices, but only 128 rows/instruction
    at ~1.4 us each -> 22 us serial for all 16 groups.
  - DMAGatherAnt (dma_gather): ~1 us + 0.34 ns/row for up to 1024 rows per
    instruction, but lives in the "mlp" Q7 ucode library whose runtime load
    costs ~9 us — and int16 indices reach only 32768 rows from a view base.

Hybrid: dispatch the mlp library reload FIRST on the Pool stream (the Q7
cluster loads it in the background — modify_pool_config responds before
load_external_libraries runs), and generate descriptors for the first groups
with INDIRECT1D on the NX sequencer while the load is in flight. Once the
library lands, two big dma_gather chunks cover the bulk at ~0.3 ns/row, and a
final INDIRECT1D group keeps the tail store small. Because each core's tokens
are sorted, a 1024-token chunk spans ~25k contiguous-ish rows < 32768, so
per-chunk windowed views table[base : base+32768] keep indices inside int16
(bases are the min over cores, identical for all 8 — one SPMD program; if an
adversarial input breaks a window, the build falls back to pure INDIRECT1D).

dma_gather quirks (from q7_kernels/extended_inst/dma_gather.cpp):
  - int16 indices, sign-extended; idx i of a chunk lives at partition i%16,
    int16 column i//16, and the 16-partition wrap is replicated to all 128
    partitions (each SWDGE queue's DSP pair streams its own window).
  - row i of a chunk lands in SBUF partition i%128, group column i//128 —
    the same layout indirect gathers and the stores use.

Raw Bass (no TileContext): the init barrier + const memsets are stripped;
engine streams only synchronize through DMA semaphores:
  - SP loads idx32 (for INDIRECT1D) then idx16 (for dma_gather), one sem
    each; stores alternate SP/ACT HWDGE rings (ssem counts all).
  - Pool waits idx32, runs the early INDIRECT1D groups (queues 0..3), waits
    idx16, issues the dma_gather chunks, then the tail INDIRECT1D group.
  - Every gather has a dedicated completion sem (cumulative counts across
    SWDGE DMAs on one sem are unsound: the 16 increments come from 16
    independently-progressing SDMA engines). Stores wait their gather's sem.
  - SP's final cumulative wait on ssem covers all stores (max total).
"""

import numpy as np

VOCAB = 50257
EMBED = 768
BATCH = 8
SEQ = 2048
N_CORES = 8
P = 128
TOK = BATCH * SEQ // N_CORES   # 2048 tokens per core
GROUPS = TOK // P              # 16 groups of 128 rows

INT16_ROWS = 32768             # rows addressable from one view base

# Hybrid layout (token positions are per-core, sorted ascending by row):
N_IND = 4                      # leading groups via INDIRECT1D (during lib load)
CHUNK_A = (N_IND * P, N_IND * P + 1024)       # [512, 1536) dma_gather
CHUNK_B = (N_IND * P + 1024, TOK - P)         # [1536, 1920) dma_gather
TAIL_G = GROUPS - 1                           # group 15 via INDIRECT1D

_cached = {}
LAST_RESULTS = None  # BassKernelResults of the most recent run (for test harness)


def _build(bases):
    """Build + compile the single-core Bass program (shared SPMD across 8
    cores). bases=(base_a, base_b) selects the hybrid program; bases=None
    builds the pure-INDIRECT1D fallback."""
    import concourse.bacc as bacc
    import concourse.bass as bass
    from concourse import library_config, mybir

    nc = bacc.Bacc(
        "TRN2",
        target_bir_lowering=False,
        debug=False,
        num_devices=N_CORES,
        num_swdge_queues=4,
    )

    # Drop the init-time const memsets and the all-engine barrier (~3.5 us):
    # nothing in this kernel reads the const APs, and the engine streams only
    # communicate through DMA semaphores which the loader zero-initializes.
    main_blk = nc.m.functions[0].blocks[0]
    removable = [
        inst
        for inst in main_blk.instructions
        if type(inst).__name__ in ("InstMemset", "InstDrain", "InstEventSemaphore")
    ]
    for inst in removable:
        main_blk.instructions.remove(inst)

    table = nc.dram_tensor(
        "table", [VOCAB, EMBED], mybir.dt.bfloat16, kind="ExternalInput"
    ).ap()
    out = nc.dram_tensor(
        "out", [GROUPS, P, EMBED], mybir.dt.bfloat16, kind="ExternalOutput"
    ).ap()

    import contextlib

    ctx = contextlib.ExitStack()
    with ctx:
        emb = ctx.enter_context(
            nc.sbuf_tensor("emb", [P, GROUPS, EMBED], mybir.dt.bfloat16)
        )
        ssem = ctx.enter_context(nc.semaphore("ssem"))

        def store(unit, g0, g1):
            """Store groups [g0, g1) once gather `unit` completed."""
            eng = nc.sync if store.k % 2 == 0 else nc.scalar
            store.k += 1
            eng.wait_ge(unit, 16)
            eng.dma_start(out[g0:g1], emb[:, g0:g1, :]).then_inc(ssem, 16)

        store.k = 0
        n_stores = 0

        def indirect(idx_sb, col, g, queue):
            sem = ctx.enter_context(nc.semaphore(f"g{g}"))
            gi = nc.gpsimd.indirect_dma_start(
                out=emb[:, g, :],
                out_offset=None,
                in_=table[:],
                in_offset=bass.IndirectOffsetOnAxis(
                    ap=idx_sb[:, col : col + 1], axis=0
                ),
            )
            if queue:
                gi.ins.queue = f"qPoolDynamic{queue}"
            gi.then_inc(sem, 16)
            return sem

        if bases is None:
            # Fallback: 16 INDIRECT1D groups, int32 indices, no library.
            idx32 = nc.dram_tensor(
                "idx32", [P, GROUPS], mybir.dt.int32, kind="ExternalInput"
            ).ap()
            idx32_sb = ctx.enter_context(
                nc.sbuf_tensor("idx32_sb", [P, GROUPS], mybir.dt.int32)
            )
            isem = ctx.enter_context(nc.semaphore("isem"))
            isem2 = ctx.enter_context(nc.semaphore("isem2"))
            with nc.allow_non_contiguous_dma(reason="idx column 0, 512B"):
                nc.sync.dma_start(idx32_sb[:, :1], idx32[:, :1]).then_inc(isem, 16)
            nc.sync.dma_start(idx32_sb[:, 1:], idx32[:, 1:]).then_inc(isem2, 16)
            nc.gpsimd.wait_ge(isem, 16)
            sems = []
            for g in range(GROUPS):
                if g == 1:
                    nc.gpsimd.wait_ge(isem2, 16)
                sems.append(indirect(idx32_sb, g, g, g % 4))
            for g, sem in enumerate(sems):
                store(sem, g, g + 1)
            n_stores = GROUPS
        else:
            base_a, base_b = bases
            n16_a = CHUNK_A[1] - CHUNK_A[0]
            n16_b = CHUNK_B[1] - CHUNK_B[0]
            cols_a = n16_a // 16
            cols_b = n16_b // 16
            idx32 = nc.dram_tensor(
                "idx32", [P, N_IND + 1], mybir.dt.int32, kind="ExternalInput"
            ).ap()
            idx16 = nc.dram_tensor(
                "idx16", [P, cols_a + cols_b], mybir.dt.int16, kind="ExternalInput"
            ).ap()
            idx32_sb = ctx.enter_context(
                nc.sbuf_tensor("idx32_sb", [P, N_IND + 1], mybir.dt.int32)
            )
            idx16_sb = ctx.enter_context(
                nc.sbuf_tensor("idx16_sb", [P, cols_a + cols_b], mybir.dt.int16)
            )
            isem32 = ctx.enter_context(nc.semaphore("isem32"))
            isem16 = ctx.enter_context(nc.semaphore("isem16"))
            asem = ctx.enter_context(nc.semaphore("asem"))
            bsem = ctx.enter_context(nc.semaphore("bsem"))

            # Q7 cluster starts pulling the mlp (dma_gather) ucode library in
            # the background; the NX sequencer keeps executing INDIRECT1D
            # meanwhile, and the first DMAGatherAnt blocks until the load's
            # internal sync barrier clears.
            nc.gpsimd.load_library(library_config.mlp)

            nc.sync.dma_start(idx32_sb[:], idx32[:]).then_inc(isem32, 16)
            nc.sync.dma_start(idx16_sb[:], idx16[:]).then_inc(isem16, 16)

            nc.gpsimd.wait_ge(isem32, 16)
            early = [indirect(idx32_sb, g, g, g % 4) for g in range(N_IND)]

            nc.gpsimd.wait_ge(isem16, 16)
            ga = nc.gpsimd.dma_gather(
                emb[:, CHUNK_A[0] // P : CHUNK_A[1] // P, :],
                table[base_a : min(base_a + INT16_ROWS, VOCAB)],
                idx16_sb[:, :cols_a],
                n16_a,
                n16_a,
                EMBED,
                queue_num=0,
            )
            ga.then_inc(asem, 16)
            gb = nc.gpsimd.dma_gather(
                emb[:, CHUNK_B[0] // P : CHUNK_B[1] // P, :],
                table[base_b : min(base_b + INT16_ROWS, VOCAB)],
                idx16_sb[:, cols_a:],
                n16_b,
                n16_b,
                EMBED,
                queue_num=1,
            )
            gb.then_inc(bsem, 16)
            # Tail group on the NX sequencer while Q7 desc-gens the chunks;
            # its store is small, trimming the end-of-kernel store tail.
            tail = indirect(idx32_sb, N_IND, TAIL_G, 2)

            for g, sem in enumerate(early):
                store(sem, g, g + 1)
            store(tail, TAIL_G, TAIL_G + 1)
            store(asem, CHUNK_A[0] // P, CHUNK_A[1] // P)
            store(bsem, CHUNK_B[0] // P, CHUNK_B[1] // P)
            n_stores = N_IND + 3

        # All stores landed (sem increments fire after last-byte receipt).
        # A cumulative wait is sound here: n_stores*16 is the maximum total.
        nc.sync.wait_ge(ssem, n_stores * 16)

    nc.compile()
    return nc


def _wrap16(vals):
    """int16 values -> [128, n/16] SWDGE idx layout for dma_gather.

    Measured HW behavior (TRN2, this exact wrap): output position p of a
    G-group chunk receives the index stored at wrap position 128*(p%G)+p//G
    (G=8 and G=3 measured; G=4 is the identity, i.e. 128*(p%4)+p//4 == the
    sim's documented order there coincides). We pre-apply the inverse
    (vals.reshape(128, G).T) so position p gathers vals[p], then lay the
    result out as: slot i -> partition i%16, int16 column i//16, with the
    16-partition wrap replicated across all 128 partitions (each SWDGE
    queue's DSP pair streams its own window)."""
    g = vals.size // P
    if g in (3, 8):          # measured: interleave-G
        vals = np.ascontiguousarray(vals.reshape(P, g).T).reshape(-1)
    else:                    # measured identity: g in (2, 4)
        assert g in (1, 2, 4), f"unmeasured dma_gather chunk shape g={g}"
    w = vals.reshape(-1, 16).T
    return np.ascontiguousarray(np.tile(w, (8, 1)))


def _ensure_axon_hooks_importable():
    """bass_utils imports antenv.axon_hooks when BASS_TRACE is set under axon;
    the agent image's antenv package lacks that module. Provide a no-op shim
    so a stray BASS_TRACE env var cannot crash the run (tracing degrades)."""
    import sys
    import types

    try:
        import antenv.axon_hooks  # noqa: F401
        return
    except ImportError:
        pass
    try:
        import antenv
    except ImportError:
        return
    mod = types.ModuleType("antenv.axon_hooks")
    _h = [None]
    mod.set_axon_ntff_profile_hook = lambda h: _h.__setitem__(0, h)
    mod.get_axon_ntff_profile_hook = lambda: _h[0]
    sys.modules["antenv.axon_hooks"] = mod
    antenv.axon_hooks = mod


def kernel(x, weight):
    global LAST_RESULTS
    import ml_dtypes

    _ensure_axon_hooks_importable()
    from concourse.bass_utils import run_bass_kernel_spmd

    wt = np.ascontiguousarray(np.asarray(weight, dtype=np.float32).T).astype(
        ml_dtypes.bfloat16
    )
    v = np.asarray(x).reshape(N_CORES, TOK).astype(np.int64)
    perms = [np.argsort(v[c], kind="stable") for c in range(N_CORES)]
    sv = np.stack([v[c][perms[c]] for c in range(N_CORES)])  # sorted rows

    # Windowed int16 bases for the dma_gather chunks: min over cores, valid
    # iff every core's chunk fits in [base, base+32768).
    base_a = int(sv[:, CHUNK_A[0]].min())
    base_b = int(sv[:, CHUNK_B[0]].min())
    ok = (
        int(sv[:, CHUNK_A[1] - 1].max()) - base_a < INT16_ROWS
        and int(sv[:, CHUNK_B[1] - 1].max()) - base_b < INT16_ROWS
    )
    bases = (base_a, base_b) if ok else None

    if bases not in _cached:
        _cached[bases] = _build(bases)
    nc = _cached[bases]

    in_maps = []
    for c in range(N_CORES):
        if bases is None:
            idx32 = np.ascontiguousarray(
                sv[c].astype(np.int32).reshape(GROUPS, P).T
            )
            in_maps.append({"table": wt, "idx32": idx32})
        else:
            ind_cols = np.concatenate([sv[c][: N_IND * P], sv[c][TAIL_G * P :]])
            idx32 = np.ascontiguousarray(
                ind_cols.astype(np.int32).reshape(N_IND + 1, P).T
            )
            idx16 = np.concatenate(
                [
                    _wrap16((sv[c][CHUNK_A[0] : CHUNK_A[1]] - base_a).astype(np.int16)),
                    _wrap16((sv[c][CHUNK_B[0] : CHUNK_B[1]] - base_b).astype(np.int16)),
                ],
                axis=1,
            )
            in_maps.append({"table": wt, "idx32": idx32, "idx16": idx16})

    res = run_bass_kernel_spmd(nc, in_maps, core_ids=list(range(N_CORES)))
    LAST_RESULTS = res

    out = np.empty((N_CORES, TOK, EMBED), dtype=np.float32)
    for c in range(N_CORES):
        rows = np.asarray(res.results[c]["out"]).reshape(TOK, EMBED)
        out[c][perms[c]] = rows.astype(np.float32)
    return out.reshape(BATCH, SEQ, EMBED)


# revision 7
# speedup vs baseline: 1.2990x; 1.2990x over previous
"""Embedding lookup (gather) on 8 Trainium2 NeuronCores — bf16 indirect DMA.

Strategy: data-parallel. The [768, 50257] fp32 table is transposed and cast to
bf16 [50257, 768] host-side (max rel err 2^-9 ~ 0.2%, well inside the 2e-2
gate) and replicated to every core's DRAM; the 16384 tokens are sharded 2048
per core (sorted by row index within each core so gathered HBM addresses are
~monotonic — better DRAM page locality; the host undoes the permutation).
Each core gathers its 2048 embedding rows from its local table copy with
indirect DMA (SWDGE) into SBUF, then streams them out bf16 to its output
shard with HWDGE stores; the host casts back to fp32. No collectives.

bf16 halves both the gather read and the store write (3.1 + 3.1 MB per core),
leaving the kernel bound by serial SWDGE descriptor generation: INDIRECT1D
is generated by Q7 pair 0 (frozen in fw) at ~1.1 us engine + ~0.3 us dispatch
per 128-row instruction, 16 instructions per core, overlapped with the DMA
transfers. Measured alternatives that do NOT win: the dma_gather extended
instruction generates descs ~2x faster but costs a ~9 us mlp-library load
that a framework drain serializes before any later Pool work (hybrid came
out at 46 us vs 34.6 us for this kernel; pure dma_gather 41 us).

Raw Bass (no TileContext, no nc.Block): all-engine barriers cost ~3-4 us each
on a ~35 us kernel, so the init barrier + const memsets are stripped from the
module and engine streams are left unsynchronized except for the DMA
semaphores that express real data dependencies:
  - SP loads the indices in two slices (column 0 first, so Q7 can start
    generating gather 0's descriptors ASAP; one sem per DMA), then stores
    each gathered group, alternating with ACT's HWDGE ring (ssem counts all).
  - Pool/GpSimd (SWDGE) waits for the indices, then issues the 16 indirect
    gathers back-to-back, batched 4-per-queue over the 4 SWDGE queues (the
    rings process FIFO per engine, so each queue's groups complete in order
    while the four rings keep several gather packets in flight per SDMA
    engine, hiding random-row HBM latency). All 16 groups are fully buffered
    in SBUF (24 KB/partition), so gathers never wait on stores.
  - Store i waits its gather's dedicated sem (gsems[i] >= 16). Cumulative
    counts across SWDGE DMAs on one sem are unsound: the 16 increments per
    DMA come from 16 independently-progressing SDMA engines.
  - SP's final cumulative wait on ssem (sound: it is the maximum total)
    covers all stores on both rings before the program retires.

NOTE: the HW indirect DMA honors only the offset AP's partition dim (<=128
indices per instruction) - a [128, 2] offset AP silently drops the second
column - so gathers are fixed at 128 rows each.
"""

import numpy as np

VOCAB = 50257
EMBED = 768
BATCH = 8
SEQ = 2048
N_CORES = 8
P = 128                      # SBUF partitions
TOK_PER_CORE = BATCH * SEQ // N_CORES   # 2048
GROUPS = TOK_PER_CORE // P              # 16 gather groups of 128 rows

_cached = {}
LAST_RESULTS = None  # BassKernelResults of the most recent run (for test harness)


def _build():
    """Build + compile the single-core Bass program (shared SPMD across 8 cores)."""
    import concourse.bacc as bacc
    import concourse.bass as bass
    from concourse import mybir

    nc = bacc.Bacc(
        "TRN2",
        target_bir_lowering=False,
        debug=False,
        num_devices=N_CORES,
        num_swdge_queues=4,
    )

    # Drop the init-time const memsets and the all-engine barrier (~3.5 us):
    # nothing in this kernel reads the const APs, and the engine streams only
    # communicate through DMA semaphores which the loader zero-initializes.
    main_blk = nc.m.functions[0].blocks[0]
    removable = [
        inst
        for inst in main_blk.instructions
        if type(inst).__name__ in ("InstMemset", "InstDrain", "InstEventSemaphore")
    ]
    for inst in removable:
        main_blk.instructions.remove(inst)

    table = nc.dram_tensor(
        "table", [VOCAB, EMBED], mybir.dt.bfloat16, kind="ExternalInput"
    ).ap()
    idx = nc.dram_tensor(
        "idx", [P, GROUPS], mybir.dt.int32, kind="ExternalInput"
    ).ap()
    out = nc.dram_tensor(
        "out", [GROUPS, P, EMBED], mybir.dt.bfloat16, kind="ExternalOutput"
    ).ap()

    import contextlib

    with contextlib.ExitStack() as ctx:
        idx_sb = ctx.enter_context(
            nc.sbuf_tensor("idx_sb", [P, GROUPS], mybir.dt.int32)
        )
        emb = ctx.enter_context(
            nc.sbuf_tensor("emb", [P, GROUPS * EMBED], mybir.dt.bfloat16)
        )
        isem = ctx.enter_context(nc.semaphore("isem"))
        isem2 = ctx.enter_context(nc.semaphore("isem2"))
        ssem = ctx.enter_context(nc.semaphore("ssem"))
        # One completion sem PER gather: a single SWDGE DMA's 16 increments
        # come from 16 independently-progressing SDMA engines, so cumulative
        # counts across DMAs on one sem do NOT imply per-DMA completion.
        gsems = [
            ctx.enter_context(nc.semaphore(f"gsem{i}")) for i in range(GROUPS)
        ]

        # SP: index load first (HWDGE - cheap descriptor gen, Q7 stays free).
        # Column 0 ships alone so Q7 can start generating gather 0's
        # descriptors at the earliest possible moment; the rest follows and
        # lands during the first generations. One sem per DMA.
        with nc.allow_non_contiguous_dma(
            reason="column 0 of the idx matrix: 128 x 4B, latency-bound either way"
        ):
            nc.sync.dma_start(idx_sb[:, :1], idx[:, :1]).then_inc(isem, 16)
        nc.sync.dma_start(idx_sb[:, 1:], idx[:, 1:]).then_inc(isem2, 16)

        # Pool/SWDGE: 16 indirect gathers, fully buffered, no store waits.
        nc.gpsimd.wait_ge(isem, 16)
        for i in range(GROUPS):
            if i == 1:
                nc.gpsimd.wait_ge(isem2, 16)
            gi = nc.gpsimd.indirect_dma_start(
                out=emb[:, i * EMBED : (i + 1) * EMBED],
                out_offset=None,
                in_=table[:],
                in_offset=bass.IndirectOffsetOnAxis(ap=idx_sb[:, i : i + 1], axis=0),
            )
            # Batch 4 consecutive gathers per SWDGE queue: same in-flight
            # depth across the 4 rings, fewer per-instruction queue switches.
            if i // 4:
                gi.ins.queue = f"qPoolDynamic{i // 4}"
            gi.then_inc(gsems[i], 16)

        # Stores: alternate the two HWDGE rings (SP=qSPDynamicHW,
        # ACT=qActDynamicHW) so more store packets are in flight per SDMA
        # engine while gather packets round-robin on the SWDGE rings.
        for i in range(GROUPS):
            eng = nc.sync if i % 2 == 0 else nc.scalar
            eng.wait_ge(gsems[i], 16)
            eng.dma_start(out[i], emb[:, i * EMBED : (i + 1) * EMBED]).then_inc(
                ssem, 16
            )

        # All stores landed (sem increments fire after last-byte receipt).
        # A cumulative wait is sound here: GROUPS*16 is the maximum total.
        nc.sync.wait_ge(ssem, GROUPS * 16)

    nc.compile()
    return nc


def _ensure_axon_hooks_importable():
    """bass_utils imports antenv.axon_hooks when BASS_TRACE is set under axon;
    the agent image's antenv package lacks that module. Provide a no-op shim
    so a stray BASS_TRACE env var cannot crash the run (tracing degrades)."""
    import sys
    import types

    try:
        import antenv.axon_hooks  # noqa: F401
        return
    except ImportError:
        pass
    try:
        import antenv
    except ImportError:
        return
    mod = types.ModuleType("antenv.axon_hooks")
    _h = [None]
    mod.set_axon_ntff_profile_hook = lambda h: _h.__setitem__(0, h)
    mod.get_axon_ntff_profile_hook = lambda: _h[0]
    sys.modules["antenv.axon_hooks"] = mod
    antenv.axon_hooks = mod


def kernel(x, weight):
    global LAST_RESULTS
    import ml_dtypes

    _ensure_axon_hooks_importable()
    from concourse.bass_utils import run_bass_kernel_spmd

    if "nc" not in _cached:
        _cached["nc"] = _build()
    nc = _cached["nc"]

    # Host-side input staging: transpose table to row-major [V, D] and cast
    # to bf16. Tokens are sharded 2048/core and sorted by row index within
    # each core (monotonic HBM addresses gather faster); perm is undone on
    # the host after the run. Group g of core c covers sorted positions
    # c*2048 + g*128 + p laid out [128 partitions, 16 groups].
    wt = np.ascontiguousarray(np.asarray(weight, dtype=np.float32).T).astype(
        ml_dtypes.bfloat16
    )
    v = np.asarray(x).reshape(N_CORES, TOK_PER_CORE).astype(np.int64)
    in_maps = []
    perms = []
    for c in range(N_CORES):
        perm = np.argsort(v[c], kind="stable")
        perms.append(perm)
        idx_c = np.ascontiguousarray(
            v[c][perm].astype(np.int32).reshape(GROUPS, P).T
        )
        in_maps.append({"table": wt, "idx": idx_c})

    res = run_bass_kernel_spmd(nc, in_maps, core_ids=list(range(N_CORES)))
    LAST_RESULTS = res

    out = np.empty((N_CORES, TOK_PER_CORE, EMBED), dtype=np.float32)
    for c in range(N_CORES):
        rows = np.asarray(res.results[c]["out"]).reshape(TOK_PER_CORE, EMBED)
        out[c][perms[c]] = rows.astype(np.float32)
    return out.reshape(BATCH, SEQ, EMBED)


# revision 9
# speedup vs baseline: 1.3122x; 1.0102x over previous
"""Embedding lookup (gather) on 8 Trainium2 NeuronCores — bf16 indirect DMA.

Strategy: data-parallel. The [768, 50257] fp32 table is transposed and cast to
bf16 [50257, 768] host-side (max rel err 2^-9 ~ 0.2%, well inside the 2e-2
gate) and replicated to every core's DRAM; the 16384 tokens are sharded 2048
per core (sorted by row index within each core so gathered HBM addresses are
~monotonic — better DRAM page locality; the host undoes the permutation).
Each core gathers its 2048 embedding rows from its local table copy with
indirect DMA (SWDGE) into SBUF, then streams them out bf16 to its output
shard with HWDGE stores; the host casts back to fp32. No collectives.

bf16 halves both the gather read and the store write (3.1 + 3.1 MB per core),
leaving the kernel bound by serial SWDGE descriptor generation: INDIRECT1D
is generated by Q7 pair 0 (frozen in fw) at ~1.1 us engine + ~0.3 us dispatch
per 128-row instruction, 16 instructions per core, overlapped with the DMA
transfers. Measured alternatives that do NOT win: the dma_gather extended
instruction generates descs ~2x faster but costs a ~9 us mlp-library load
that a framework drain serializes before any later Pool work (hybrid came
out at 46 us vs 34.6 us for this kernel; pure dma_gather 41 us).

Raw Bass (no TileContext, no nc.Block): all-engine barriers cost ~3-4 us each
on a ~35 us kernel, so the init barrier + const memsets are stripped from the
module and engine streams are left unsynchronized except for the DMA
semaphores that express real data dependencies:
  - SP loads the indices in two slices (column 0 first, so Q7 can start
    generating gather 0's descriptors ASAP; one sem per DMA), then stores
    each gathered group, alternating with ACT's HWDGE ring (ssem counts all).
  - Pool/GpSimd (SWDGE) waits for the indices, then issues the 16 indirect
    gathers back-to-back, round-robin over 4 SWDGE queues so each SDMA
    engine keeps several gather packets in flight (hides random-row HBM
    latency). All 16 groups are fully buffered in SBUF (24 KB/partition),
    so gathers never wait on stores.
  - Store i waits its gather's dedicated sem (gsems[i] >= 16). Cumulative
    counts across SWDGE DMAs on one sem are unsound: the 16 increments per
    DMA come from 16 independently-progressing SDMA engines.
  - SP's final cumulative wait on ssem (sound: it is the maximum total)
    covers all stores on both rings before the program retires.

NOTE: the HW indirect DMA honors only the offset AP's partition dim (<=128
indices per instruction) - a [128, 2] offset AP silently drops the second
column - so gathers are fixed at 128 rows each.
"""

import numpy as np

VOCAB = 50257
EMBED = 768
BATCH = 8
SEQ = 2048
N_CORES = 8
P = 128                      # SBUF partitions
TOK_PER_CORE = BATCH * SEQ // N_CORES   # 2048
GROUPS = TOK_PER_CORE // P              # 16 gather groups of 128 rows

_cached = {}
LAST_RESULTS = None  # BassKernelResults of the most recent run (for test harness)


def _build():
    """Build + compile the single-core Bass program (shared SPMD across 8 cores)."""
    import concourse.bacc as bacc
    import concourse.bass as bass
    from concourse import mybir

    nc = bacc.Bacc(
        "TRN2",
        target_bir_lowering=False,
        debug=False,
        num_devices=N_CORES,
        num_swdge_queues=4,
    )

    # Drop the init-time const memsets and the all-engine barrier (~3.5 us):
    # nothing in this kernel reads the const APs, and the engine streams only
    # communicate through DMA semaphores which the loader zero-initializes.
    main_blk = nc.m.functions[0].blocks[0]
    removable = [
        inst
        for inst in main_blk.instructions
        if type(inst).__name__ in ("InstMemset", "InstDrain", "InstEventSemaphore")
    ]
    for inst in removable:
        main_blk.instructions.remove(inst)

    table = nc.dram_tensor(
        "table", [VOCAB, EMBED], mybir.dt.bfloat16, kind="ExternalInput"
    ).ap()
    idx = nc.dram_tensor(
        "idx", [P, GROUPS], mybir.dt.int32, kind="ExternalInput"
    ).ap()
    out = nc.dram_tensor(
        "out", [GROUPS, P, EMBED], mybir.dt.bfloat16, kind="ExternalOutput"
    ).ap()

    import contextlib

    with contextlib.ExitStack() as ctx:
        idx_sb = ctx.enter_context(
            nc.sbuf_tensor("idx_sb", [P, GROUPS], mybir.dt.int32)
        )
        emb = ctx.enter_context(
            nc.sbuf_tensor("emb", [P, GROUPS * EMBED], mybir.dt.bfloat16)
        )
        isem = ctx.enter_context(nc.semaphore("isem"))
        isem2 = ctx.enter_context(nc.semaphore("isem2"))
        ssem = ctx.enter_context(nc.semaphore("ssem"))
        # One completion sem PER gather: a single SWDGE DMA's 16 increments
        # come from 16 independently-progressing SDMA engines, so cumulative
        # counts across DMAs on one sem do NOT imply per-DMA completion.
        gsems = [
            ctx.enter_context(nc.semaphore(f"gsem{i}")) for i in range(GROUPS)
        ]

        # SP: index load first (HWDGE - cheap descriptor gen, Q7 stays free).
        # Column 0 ships alone so Q7 can start generating gather 0's
        # descriptors at the earliest possible moment; the rest follows and
        # lands during the first generations. One sem per DMA.
        with nc.allow_non_contiguous_dma(
            reason="column 0 of the idx matrix: 128 x 4B, latency-bound either way"
        ):
            nc.sync.dma_start(idx_sb[:, :1], idx[:, :1]).then_inc(isem, 16)
        nc.sync.dma_start(idx_sb[:, 1:], idx[:, 1:]).then_inc(isem2, 16)

        # Pool/SWDGE: 16 indirect gathers, fully buffered, no store waits.
        nc.gpsimd.wait_ge(isem, 16)
        for i in range(GROUPS):
            if i == 1:
                nc.gpsimd.wait_ge(isem2, 16)
            gi = nc.gpsimd.indirect_dma_start(
                out=emb[:, i * EMBED : (i + 1) * EMBED],
                out_offset=None,
                in_=table[:],
                in_offset=bass.IndirectOffsetOnAxis(ap=idx_sb[:, i : i + 1], axis=0),
            )
            # Round-robin the SWDGE queues so each SDMA engine holds gather
            # packets from several rings and keeps more outstanding HBM
            # reads in flight (hides random-row latency); per-queue FIFO
            # also completes early groups first, so stores start sooner.
            if i % 4:
                gi.ins.queue = f"qPoolDynamic{i % 4}"
            gi.then_inc(gsems[i], 16)

        # Stores: alternate the two HWDGE rings (SP=qSPDynamicHW,
        # ACT=qActDynamicHW) so more store packets are in flight per SDMA
        # engine while gather packets round-robin on the SWDGE rings.
        for i in range(GROUPS):
            eng = nc.sync if i % 2 == 0 else nc.scalar
            eng.wait_ge(gsems[i], 16)
            eng.dma_start(out[i], emb[:, i * EMBED : (i + 1) * EMBED]).then_inc(
                ssem, 16
            )

        # All stores landed (sem increments fire after last-byte receipt).
        # A cumulative wait is sound here: GROUPS*16 is the maximum total.
        nc.sync.wait_ge(ssem, GROUPS * 16)

    nc.compile()
    return nc


def _ensure_axon_hooks_importable():
    """bass_utils imports antenv.axon_hooks when BASS_TRACE is set under axon;
    the agent image's antenv package lacks that module. Provide a no-op shim
    so a stray BASS_TRACE env var cannot crash the run (tracing degrades)."""
    import sys
    import types

    try:
        import antenv.axon_hooks  # noqa: F401
        return
    except ImportError:
        pass
    try:
        import antenv
    except ImportError:
        return
    mod = types.ModuleType("antenv.axon_hooks")
    _h = [None]
    mod.set_axon_ntff_profile_hook = lambda h: _h.__setitem__(0, h)
    mod.get_axon_ntff_profile_hook = lambda: _h[0]
    sys.modules["antenv.axon_hooks"] = mod
    antenv.axon_hooks = mod


def kernel(x, weight):
    global LAST_RESULTS
    import ml_dtypes

    _ensure_axon_hooks_importable()
    from concourse.bass_utils import run_bass_kernel_spmd

    if "nc" not in _cached:
        _cached["nc"] = _build()
    nc = _cached["nc"]

    # Host-side input staging: transpose table to row-major [V, D] and cast
    # to bf16. Tokens are sharded 2048/core and sorted by row index within
    # each core (monotonic HBM addresses gather faster); perm is undone on
    # the host after the run. Group g of core c covers sorted positions
    # c*2048 + g*128 + p laid out [128 partitions, 16 groups].
    wt = np.ascontiguousarray(np.asarray(weight, dtype=np.float32).T).astype(
        ml_dtypes.bfloat16
    )
    v = np.asarray(x).reshape(N_CORES, TOK_PER_CORE).astype(np.int64)
    in_maps = []
    perms = []
    for c in range(N_CORES):
        perm = np.argsort(v[c], kind="stable")
        perms.append(perm)
        idx_c = np.ascontiguousarray(
            v[c][perm].astype(np.int32).reshape(GROUPS, P).T
        )
        in_maps.append({"table": wt, "idx": idx_c})

    res = run_bass_kernel_spmd(nc, in_maps, core_ids=list(range(N_CORES)))
    LAST_RESULTS = res

    out = np.empty((N_CORES, TOK_PER_CORE, EMBED), dtype=np.float32)
    for c in range(N_CORES):
        rows = np.asarray(res.results[c]["out"]).reshape(TOK_PER_CORE, EMBED)
        out[c][perms[c]] = rows.astype(np.float32)
    return out.reshape(BATCH, SEQ, EMBED)
